# revision 35
# baseline (speedup 1.0000x reference)
"""Trainium2 Bass kernel for nn_ConvM_Layer (episode covariance similarity).

Math reformulation (exact):
  cov      = S_c S_c^T / (hw-1)  with S_c the per-(t,way) centered support (c x 500)
  cov_sim  = q^T cov q = ||S_c^T q||^2 / (hw-1)  >= 0   (PSD quadratic form)
  => LeakyReLU is the identity, and
  score[t,q,w] = sum_p conv_w[p]/(hw-1) * ||S_c^T (q_p - qbar)||^2 + conv_b

Sharding: 8 cores = (t in 0..3) x (wq half in 0..1); wq padded 75 -> 76 = 2*38.
Each core computes its (t, half) shard independently; host gathers.

Inputs are pre-transposed on host to channel-major so every DMA partition-row
is one contiguous burst. DMA issue is split across the two HWDGE engines
(Sync, Scalar); a short chain of dummy warm-up matmuls keeps the PE HAM
clock-gate at 2.4 GHz so the real matmuls never run cold.
"""

from contextlib import ExitStack

import numpy as np

import concourse.bass as bass
import concourse.tile as tile
from concourse import bacc, mybir
from concourse.bass_utils import run_bass_kernel_spmd

# Problem shape (hardcoded per contract)
T, WQ, C, H, W = 4, 75, 640, 10, 10
HW = H * W                 # 100
WAY, SHOT = 5, 5
M = SHOT * HW              # 500 support samples per way
WQP = 76                   # padded query count (divisible by 2)
WQH = WQP // 2             # 38 queries per core
NQ = WQH * HW              # 3800 query spatial columns per core
CT = C // 128              # 5 contraction tiles
N_CORES = 8
CHUNKS = [(0, 8), (8, 8), (16, 8), (24, 8), (32, 6)]  # (q0, nq) pipeline chunks
QCH = 8                    # max chunk size
N_WARM = 60                # dummy matmuls that pre-warm the PE clock gate

F32 = mybir.dt.float32
# float32r streams at ~1 cyc/row (vs 4 for f32); ~1.5e-4 rel err end-to-end.
DT_MM = mybir.dt.float32r

_CACHE: dict = {}


def _emit_warmup(nc, warm_p, wps_p, n):
    wsrc = warm_p.tile([128, 512], mybir.dt.bfloat16, name="wsrc")
    nc.vector.memset(wsrc[:], 0.0)
    wps = wps_p.tile([128, 512], F32, name="wpsum")
    for _ in range(n):
        nc.tensor.matmul(wps[:], wsrc[:, :128], wsrc[:], start=True, stop=True)


def _center(nc, out_ap, in_ap, shape_chw, stat_p, tag, block=HW, eng=None):
    """in_ap: [128, ct, n*block] f32; out_ap: same-shape DT_MM tile. Subtract
    the mean of each `block`-sized group."""
    X = mybir.AxisListType.X
    if eng is None:
        eng = nc.vector
    ctdim, n = shape_chw
    v4 = in_ap.rearrange("c t (q h) -> c t q h", h=block)
    mean = stat_p.tile([128, ctdim * n], F32, name=f"mean_{tag}", tag="mean")
    mv = mean[:].rearrange("c (t q) -> c t q", t=ctdim)
    nc.vector.reduce_sum(mv, v4, axis=X)
    nc.vector.tensor_scalar_mul(mean[:], mean[:], 1.0 / block)
    out = out_ap.rearrange("c t (q h) -> c t q h", h=block)
    eng.tensor_sub(out, v4, mv.broadcast_to((128, ctdim, n, block)))


def _kernel_body_v3(ctx: ExitStack, tc: tile.TileContext, q_d, s_d, w_d, o_d):
    nc = tc.nc

    s_p = ctx.enter_context(tc.tile_pool(name="sp", bufs=1))
    sraw_p = ctx.enter_context(tc.tile_pool(name="sraw", bufs=2))
    qraw_p = ctx.enter_context(tc.tile_pool(name="qraw", bufs=2))
    qc_p = ctx.enter_context(tc.tile_pool(name="qcp", bufs=len(CHUNKS)))
    stat_p = ctx.enter_context(tc.tile_pool(name="stat", bufs=10))
    trash_p = ctx.enter_context(tc.tile_pool(name="trash", bufs=1))
    lcs_p = ctx.enter_context(tc.tile_pool(name="lcs", bufs=1))
    w_p = ctx.enter_context(tc.tile_pool(name="wgt", bufs=1))
    osb_p = ctx.enter_context(tc.tile_pool(name="osb", bufs=1))
    warm_p = ctx.enter_context(tc.tile_pool(name="warm", bufs=1))
    ps_p = ctx.enter_context(tc.tile_pool(name="ps", bufs=5, space="PSUM"))
    wps_p = ctx.enter_context(tc.tile_pool(name="wps", bufs=1, space="PSUM"))
    ops_p = ctx.enter_context(tc.tile_pool(name="ops", bufs=1, space="PSUM"))

    _emit_warmup(nc, warm_p, wps_p, N_WARM)

    # ---- support: way0 first (fast start), ways 1-4 as per-ct slabs ----
    # raw staging recycles; centered f32r tiles persist.
    s0r = sraw_p.tile([128, CT, M], F32, name="s0r", tag="sraw", bufs=1)
    nc.sync.dma_start(s0r[:], s_d[:, 0:M].rearrange("(t c) m -> c t m", c=128))

    qst, qcv = [None] * len(CHUNKS), [None] * len(CHUNKS)

    def q_load(ki):
        q0, nq = CHUNKS[ki]
        stg = qraw_p.tile([128, CT, QCH * HW], F32, name=f"qst{ki}", tag="qst",
                          bufs=1)
        eng = nc.sync
        eng.dma_start(
            stg[:, :, :nq * HW],
            q_d[:, q0 * HW:(q0 + nq) * HW].rearrange("(t c) n -> c t n", c=128),
        )
        qst[ki] = stg

    def q_center(ki):
        q0, nq = CHUNKS[ki]
        qc = qc_p.tile([128, CT, QCH * HW], DT_MM, name=f"qc{ki}", tag="qc")
        _center(nc, qc[:, :, :nq * HW], qst[ki][:, :, :nq * HW],
                (CT, nq), stat_p, f"q{ki}", eng=nc.vector)
        qcv[ki] = qc

    q_load(0)
    # S centering rides GpSimd so it never blocks Q centering on DVE
    s0c = s_p.tile([128, CT, M], DT_MM, name="s0c")
    _center(nc, s0c[:], s0r[:], (CT, 1), stat_p, "s0", block=M, eng=nc.gpsimd)
    q_center(0)

    ssl_raw = []
    for ct in range(CT):
        sr = sraw_p.tile([128, WAY - 1, M], F32, name=f"ssl{ct}r", tag="sslr")
        nc.sync.dma_start(
            sr[:],
            s_d[ct * 128:(ct + 1) * 128, M:].rearrange("c (w m) -> c w m", m=M),
        )
        ssl_raw.append(sr)
    q_load(1)
    sslc = []
    for ct in range(CT):
        sc = s_p.tile([128, WAY - 1, M], DT_MM, name=f"ssl{ct}c")
        _center(nc, sc[:], ssl_raw[ct][:], (WAY - 1, 1), stat_p, f"ssl{ct}",
                block=M, eng=nc.gpsimd)
        sslc.append(sc)
    q_center(1)
    for ki in range(2, len(CHUNKS)):
        q_load(ki)
        q_center(ki)

    w_sb = w_p.tile([HW, 1], F32)
    nc.sync.dma_start(w_sb[:], w_d[:])

    def s_rhs(wy, ct):
        if wy == 0:
            return s0c[:, ct, :]
        return sslc[ct][:, wy - 1, :]

    # ---- main: P = S_c^T Q_q per (way, query); cs col = rowwise ||.||^2 ----
    # way-outer: the way-0 sweep only needs s0c + q chunks, so it runs while
    # GpSimd is still centering ways 1-4.
    lcs = lcs_p.tile([HW, WAY * WQH], F32)
    for wy in range(WAY):
        for ki, (q0, nq) in enumerate(CHUNKS):
            for ql in range(nq):
                ps = ps_p.tile([HW, M], F32)
                for ct in range(CT):
                    nc.tensor.matmul(
                        ps[:],
                        qcv[ki][:, ct, ql * HW:(ql + 1) * HW],
                        s_rhs(wy, ct),
                        start=(ct == 0),
                        stop=(ct == CT - 1),
                    )
                trash = trash_p.tile([HW, M], F32)
                col = wy * WQH + (q0 + ql)
                nc.scalar.activation(
                    trash[:], ps[:], mybir.ActivationFunctionType.Square,
                    accum_out=lcs[:, col:col + 1],
                )

    # ---- score row = conv_w^T @ lcs  -> [1, WAY*WQH] ----
    ops = ops_p.tile([1, WAY * WQH], F32)
    nc.tensor.matmul(ops[:], w_sb[:], lcs[:], start=True, stop=True)
    osb = osb_p.tile([1, WAY * WQH], F32)
    nc.scalar.copy(osb[:], ops[:])
    nc.sync.dma_start(o_d[:], osb[:])


def _build():
    key = "nc"
    if key in _CACHE:
        return _CACHE[key]
    nc = bacc.Bacc(
        "TRN2", target_bir_lowering=False, debug=False, num_devices=N_CORES
    )
    q_d = nc.dram_tensor("q", [C, NQ], F32, kind="ExternalInput").ap()
    s_d = nc.dram_tensor("s", [C, WAY * M], F32, kind="ExternalInput").ap()
    w_d = nc.dram_tensor("w", [HW, 1], F32, kind="ExternalInput").ap()
    o_d = nc.dram_tensor("out", [1, WAY * WQH], F32, kind="ExternalOutput").ap()
    with tile.TileContext(nc) as tc:
        with ExitStack() as ctx:
            _kernel_body_v3(ctx, tc, q_d, s_d, w_d, o_d)
    nc.compile()
    _CACHE[key] = nc
    return nc


def make_in_maps(query_feat, support_feat, conv_w):
    q = np.asarray(query_feat, dtype=np.float32).reshape(T, WQ, C, HW)
    s = np.asarray(support_feat, dtype=np.float32).reshape(T, WAY * SHOT, C, HW)
    w_col = np.ascontiguousarray(
        (np.asarray(conv_w, dtype=np.float32)[0, 0] / (HW - 1)).reshape(HW, 1)
    )
    # channel-major transposes so every DMA partition-row is contiguous
    qt = np.zeros((T, C, WQP * HW), dtype=np.float32)
    qt[:, :, :WQ * HW] = q.transpose(0, 2, 1, 3).reshape(T, C, WQ * HW)
    st = np.ascontiguousarray(s.transpose(0, 2, 1, 3).reshape(T, C, WAY * M))
    in_maps = []
    for core in range(N_CORES):
        ti, half = core // 2, core % 2
        in_maps.append({
            "q": np.ascontiguousarray(qt[ti, :, half * NQ:(half + 1) * NQ]),
            "s": st[ti],
            "w": w_col,
        })
    return in_maps


LAST_RESULT = None  # set by kernel(); lets a harness read exec_time_ns/profile


def kernel(query_feat, support_feat, conv_w, conv_b):
    global LAST_RESULT
    nc = _build()
    in_maps = make_in_maps(query_feat, support_feat, conv_w)
    res = run_bass_kernel_spmd(nc, in_maps, list(range(N_CORES)))
    LAST_RESULT = res
    score = np.empty((T, WQP, WAY), dtype=np.float32)
    for core in range(N_CORES):
        ti, half = core // 2, core % 2
        row = res.results[core]["out"][0]  # [WAY*WQH]
        score[ti, half * WQH:(half + 1) * WQH, :] = row.reshape(WAY, WQH).T
    out = score[:, :WQ, :] + np.asarray(conv_b, dtype=np.float32)[0]
    return np.ascontiguousarray(out)
